# revision 1
# baseline (speedup 1.0000x reference)
"""Trainium2 Bass kernel for DeepKernelRegressionModel.

Math (per core, X sharded by rows across 8 cores):
  Xf = MLP(X), Yf = MLP(Y)                        (3-layer relu MLP, H=32)
  K[i,m] = exp(-|Xf_i - Yf_m|^2 / 2)
         = exp(Xf_i . Yf_m - |Xf_i|^2/2 - |Yf_m|^2/2)
  out = (K @ Y_target) / (K @ 1)

Everything is fused: the exponent is produced by ONE tensor-engine matmul
with an augmented contraction dim (K=34):
  lhsT rows 0-31 = Yf^T, row 32 = 1,       row 33 = -|Yf|^2/2
  rhs  rows 0-31 = Xf^T, row 32 = -|Xf|^2/2, row 33 = 1
in the transposed orientation G'[m, i], so that the second matmul
  acc[t, i] += Z_chunk^T @ exp(G')      with Z = [Y_target, 1]
contracts over m (the partition dim) with no transposes of the big
exp matrix. A final tiny transpose + reciprocal produces out[i, t].

The MLPs run in the transposed orientation (features on partitions) with
4-way tile_position stacking so relu ops use all 128 partitions.
"""

import os
import numpy as np
from contextlib import ExitStack

import concourse.bass as bass
import concourse.tile as tile
from concourse import bacc, mybir

FP = mybir.dt.float32
FPR = mybir.dt.float32r
AF = mybir.ActivationFunctionType

D, H, T = 64, 32, 8
TZ = T + 1  # Y_target columns + ones column
ZP = 32     # Z padded to 32 cols so mm2 fully writes its PSUM stripes
N_CORES = 8


def _split_matmul_waits(nc):
    """Walrus's S3_LW lowering for self-loading (4-byte) matmuls supports only
    one sync-wait command. Move multi-waits onto a PE sequencer NoOp placed
    right before the matmul — the in-order NX applies them to the stream."""
    import bass_rust

    k = 0
    for fn in nc.m.functions:
        for blk in fn.blocks:
            out = []
            for inst in blk.instructions:
                si = inst.sync_info
                if (type(inst).__name__ == "InstMatmult" and si is not None
                        and si.on_wait and len(si.on_wait) >= 2):
                    waits = list(si.on_wait)
                    for w in waits[:-1]:
                        nop = mybir.InstNoOp(name=f"I-mmwait-{k}", ins=[],
                                             outs=[])
                        k += 1
                        nop.engine = inst.engine
                        nop.sync_info = bass_rust.SyncInfo(
                            on_wait=[w], on_update=[])
                        out.append(nop)
                    inst.sync_info = bass_rust.SyncInfo(
                        on_wait=[waits[-1]], on_update=list(si.on_update))
                out.append(inst)
            blk.instructions = out


def build_nc(n_sh, m_total, use_f32r=True, exp_group=3, split_waits=True):
    """Build the Bass program for one core (SPMD: same program, all cores).

    n_sh: rows of X handled by this core. m_total: rows of Y (full).
    """
    assert n_sh % 512 == 0 and m_total % 2048 == 0
    MT = m_total // 128       # number of 128-row m-tiles
    NCH = m_total // 512      # number of 512-wide m-chunks (MLP)
    XG = n_sh // 4            # X stacked-chunk width
    IC = n_sh // 512          # i-chunks
    ICW = 512

    def r(ap):
        return ap.bitcast(FPR) if use_f32r else ap

    nc = bacc.Bacc("TRN2", target_bir_lowering=False, debug=False,
                   num_devices=N_CORES)

    Xd = nc.dram_tensor("X", [n_sh, D], FP, kind="ExternalInput").ap()
    Yd = nc.dram_tensor("Y", [m_total, D], FP, kind="ExternalInput").ap()
    Zd = nc.dram_tensor("Zm", [m_total, ZP], FP, kind="ExternalInput").ap()
    W1d = nc.dram_tensor("W1", [D, H], FP, kind="ExternalInput").ap()
    W2d = nc.dram_tensor("W2", [H, H], FP, kind="ExternalInput").ap()
    W3d = nc.dram_tensor("W3", [H, H], FP, kind="ExternalInput").ap()
    Bd = nc.dram_tensor("Bs", [128, 3], FP, kind="ExternalInput").ap()
    Id = nc.dram_tensor("ident", [128, 128], FP, kind="ExternalInput").ap()
    NHd = nc.dram_tensor("neghalf", [128, 32], FP, kind="ExternalInput").ap()
    ORd = nc.dram_tensor("onesrow", [1, m_total], FP, kind="ExternalInput").ap()
    OUTd = nc.dram_tensor("out", [n_sh, T], FP, kind="ExternalOutput").ap()

    with tile.TileContext(nc) as tc, ExitStack() as ctx:
        const = ctx.enter_context(tc.tile_pool(name="const", bufs=1))
        big = ctx.enter_context(tc.tile_pool(name="big", bufs=1))
        scr = ctx.enter_context(tc.tile_pool(name="scr", bufs=1))

        w1s = const.tile([D, H], FP)
        nc.sync.dma_start(w1s[:], W1d[:])
        w2s = const.tile([128, H], FP)
        w3s = const.tile([128, H], FP)
        for g in range(4):
            nc.sync.dma_start(w2s[32 * g:32 * g + 32, :], W2d[:])
            nc.sync.dma_start(w3s[32 * g:32 * g + 32, :], W3d[:])
        bs = const.tile([128, 3], FP)
        nc.sync.dma_start(bs[:], Bd[:])
        ident = const.tile([128, 128], FP)
        nc.sync.dma_start(ident[:], Id[:])
        nh = const.tile([128, 32], FP)
        nc.sync.dma_start(nh[:], NHd[:])
        zt = const.tile([128, MT * ZP], FP)
        nc.sync.dma_start(
            r(zt.rearrange("p (t c) -> p t c", c=ZP)),
            r(Zd.rearrange("(t p) c -> p t c", p=128)),
        )

        # persistent big tensors
        yT = big.tile([D, m_total], FP)      # Y^T
        xT = big.tile([D, n_sh], FP)         # X^T
        yft = big.tile([128, m_total], FP)   # rows 0-33 aug A, 64-97 aug B
        xft = big.tile([128, n_sh], FP)

        # ---------------- phase A: transposes (PE) ----------------
        with (
            tc.tile_pool(name="tp_psum", bufs=2, space="PSUM") as tpp,
            tc.tile_pool(name="ytile", bufs=4) as ytp,
        ):
            n_ych = (MT + 7) // 8
            for c in range(n_ych):
                ts = list(range(8 * c, min(8 * c + 8, MT)))
                tp = tpp.tile([D, 128 * len(ts)], FP, tag="tp")
                for k, mt in enumerate(ts):
                    ytile = ytp.tile([128, D], FP, tag="yt")
                    nc.sync.dma_start(ytile[:], Yd[128 * mt:128 * mt + 128, :])
                    nc.tensor.transpose(tp[:, 128 * k:128 * k + 128],
                                        ytile[:], ident[:])
                nc.vector.tensor_copy(
                    yT[:, 1024 * c:1024 * c + 128 * len(ts)], tp[:])
            n_xch = (n_sh // 128 + 7) // 8
            for c in range(n_xch):
                ts = list(range(8 * c, min(8 * c + 8, n_sh // 128)))
                tp = tpp.tile([D, 128 * len(ts)], FP, tag="tp")
                for k, mt in enumerate(ts):
                    xtile = ytp.tile([128, D], FP, tag="yt")
                    nc.sync.dma_start(xtile[:], Xd[128 * mt:128 * mt + 128, :])
                    nc.tensor.transpose(tp[:, 128 * k:128 * k + 128],
                                        xtile[:], ident[:])
                nc.vector.tensor_copy(
                    xT[:, 1024 * c:1024 * c + 128 * len(ts)], tp[:])

        # ---------------- phase B: Y MLP (stacked 4x) ----------------
        # chunk ch (512 m's) -> partition group cg = ch%4, col chunk cc = ch//4
        CCY = NCH // 4
        yfp = ctx.enter_context(tc.tile_pool(name="yf_pool", bufs=1))
        with (
            tc.tile_pool(name="mlp_psum", bufs=2, space="PSUM") as mpp,
            tc.tile_pool(name="acts", bufs=2) as actp,
        ):
            h1p = mpp.tile([128, 512 * CCY], FP, tag="hp")
            for ch in range(NCH):
                cg, cc = ch % 4, ch // 4
                nc.tensor.matmul(h1p[32 * cg:32 * cg + 32, 512 * cc:512 * cc + 512],
                                 lhsT=w1s[:], rhs=yT[:, 512 * ch:512 * ch + 512],
                                 start=True, stop=True,
                                 skip_group_check=True,
                                 tile_position=(0, 32 * cg))
            h1s = actp.tile([128, 512 * CCY], FP, tag="hs")
            for cc in range(CCY):
                nc.scalar.activation(h1s[:, 512 * cc:512 * cc + 512],
                                     h1p[:, 512 * cc:512 * cc + 512],
                                     AF.Relu, bias=bs[:, 0:1])
            h2p = mpp.tile([128, 512 * CCY], FP, tag="hp")
            for ch in range(NCH):
                cg, cc = ch % 4, ch // 4
                nc.tensor.matmul(h2p[32 * cg:32 * cg + 32, 512 * cc:512 * cc + 512],
                                 tile_position=(32 * cg, 32 * cg),
                                 lhsT=w2s[32 * cg:32 * cg + 32, :],
                                 rhs=h1s[32 * cg:32 * cg + 32, 512 * cc:512 * cc + 512],
                                 start=True, stop=True,
                                 skip_group_check=True)
            h2s = actp.tile([128, 512 * CCY], FP, tag="hs")
            for cc in range(CCY):
                nc.scalar.activation(h2s[:, 512 * cc:512 * cc + 512],
                                     h2p[:, 512 * cc:512 * cc + 512],
                                     AF.Relu, bias=bs[:, 1:2])
            h3p = mpp.tile([128, 512 * CCY], FP, tag="hp")
            for ch in range(NCH):
                cg, cc = ch % 4, ch // 4
                nc.tensor.matmul(h3p[32 * cg:32 * cg + 32, 512 * cc:512 * cc + 512],
                                 tile_position=(32 * cg, 32 * cg),
                                 lhsT=w3s[32 * cg:32 * cg + 32, :],
                                 rhs=h2s[32 * cg:32 * cg + 32, 512 * cc:512 * cc + 512],
                                 start=True, stop=True,
                                 skip_group_check=True)
            yfs = yfp.tile([128, 512 * CCY], FP, tag="yfs")
            sqy = yfp.tile([128, 512 * CCY], FP, tag="sqy")
            for cc in range(CCY):
                nc.scalar.activation(r(yfs[:, 512 * cc:512 * cc + 512]),
                                     h3p[:, 512 * cc:512 * cc + 512],
                                     AF.Relu, bias=bs[:, 2:3])
                nc.vector.tensor_mul(sqy[:, 512 * cc:512 * cc + 512],
                                     yfs[:, 512 * cc:512 * cc + 512],
                                     yfs[:, 512 * cc:512 * cc + 512])
            # assemble yft rows 0-31 (flat layout)
            for ch in range(NCH):
                cg, cc = ch % 4, ch // 4
                nc.sync.dma_start(r(yft[0:32, 512 * ch:512 * ch + 512]),
                                  r(yfs[32 * cg:32 * cg + 32, 512 * cc:512 * cc + 512]))
            nc.sync.dma_start(r(yft[32:33, :]), r(ORd[:]))  # ones row

        # ---------------- phase C: norms + X MLP ----------------
        with (
            tc.tile_pool(name="ynp", bufs=2, space="PSUM") as ynpp,
            tc.tile_pool(name="xnp", bufs=1, space="PSUM") as xnpp,
        ):
            # ynorm row: -|Yf_m|^2/2 for every m, laid out on partitions
            # {0,32,64,96} x 1024 cols per psum tile (one tile per 4096 m)
            n_yn = (NCH + 7) // 8
            for a in range(n_yn):
                chs = list(range(8 * a, min(8 * a + 8, NCH)))
                ynp = ynpp.tile([128, 1024], FP, tag="ynp")
                for chl, ch in enumerate(chs):
                    cg, cc = ch % 4, ch // 4
                    prow, pcol = 32 * (chl // 2), 512 * (chl % 2)
                    nc.tensor.matmul(ynp[prow:prow + 32, pcol:pcol + 512],
                                     tile_position=(32 * cg, prow),
                                     lhsT=nh[32 * cg:32 * cg + 32, :],
                                     rhs=sqy[32 * cg:32 * cg + 32,
                                             512 * cc:512 * cc + 512],
                                     start=True, stop=True,
                                     skip_group_check=True)
                yns = scr.tile([128, 1024], FP, tag="yns")
                nrow = 32 * ((len(chs) + 1) // 2)
                nc.vector.tensor_copy(r(yns[0:nrow, :]), ynp[0:nrow, :])
                for k in range(len(chs) // 2):
                    nc.sync.dma_start(
                        r(yft[33:34, 4096 * a + 1024 * k:4096 * a + 1024 * k + 1024]),
                        r(yns[32 * k:32 * k + 1, :]))
            # duplicate augmented block to partitions 64-97 (row group B)
            for sg in range(4):
                seg = m_total // 4
                nc.sync.dma_start(r(yft[64:98, seg * sg:seg * sg + seg]),
                                  r(yft[0:34, seg * sg:seg * sg + seg]))

            # ---- X MLP (4 chunks of XG cols, stacked) ----
            hx1 = xnpp.tile([128, XG], FP, tag="hx")
            for ch in range(4):
                nc.tensor.matmul(hx1[32 * ch:32 * ch + 32, :],
                                 tile_position=(0, 32 * ch),
                                 lhsT=w1s[:],
                                 rhs=xT[:, XG * ch:XG * ch + XG],
                                 start=True, stop=True,
                                 skip_group_check=True)
            hx1s = scr.tile([128, XG], FP, tag="hxs1")
            nc.scalar.activation(hx1s[:], hx1[:], AF.Relu, bias=bs[:, 0:1])
            hx2 = xnpp.tile([128, XG], FP, tag="hx")
            for ch in range(4):
                nc.tensor.matmul(hx2[32 * ch:32 * ch + 32, :],
                                 tile_position=(32 * ch, 32 * ch),
                                 lhsT=w2s[32 * ch:32 * ch + 32, :],
                                 rhs=hx1s[32 * ch:32 * ch + 32, :],
                                 start=True, stop=True,
                                 skip_group_check=True)
            hx2s = scr.tile([128, XG], FP, tag="hxs2")
            nc.scalar.activation(hx2s[:], hx2[:], AF.Relu, bias=bs[:, 1:2])
            hx3 = xnpp.tile([128, XG], FP, tag="hx")
            for ch in range(4):
                nc.tensor.matmul(hx3[32 * ch:32 * ch + 32, :],
                                 tile_position=(32 * ch, 32 * ch),
                                 lhsT=w3s[32 * ch:32 * ch + 32, :],
                                 rhs=hx2s[32 * ch:32 * ch + 32, :],
                                 start=True, stop=True,
                                 skip_group_check=True)
            xfs = scr.tile([128, XG], FP, tag="xfs")
            nc.scalar.activation(r(xfs[:]), hx3[:], AF.Relu, bias=bs[:, 2:3])
            sqx = scr.tile([128, XG], FP, tag="sqx")
            nc.vector.tensor_mul(sqx[:], xfs[:], xfs[:])
            for ch in range(4):
                nc.sync.dma_start(r(xft[0:32, XG * ch:XG * ch + XG]),
                                  r(xfs[32 * ch:32 * ch + 32, :]))
            nc.sync.dma_start(r(xft[33:34, :]), r(ORd[0:1, 0:n_sh]))  # ones row
            xnp = xnpp.tile([128, XG], FP, tag="xnp")
            for ch in range(4):
                nc.tensor.matmul(xnp[32 * ch:32 * ch + 32, :],
                                 tile_position=(32 * ch, 32 * ch),
                                 lhsT=nh[32 * ch:32 * ch + 32, :],
                                 rhs=sqx[32 * ch:32 * ch + 32, :],
                                 start=True, stop=True,
                                 skip_group_check=True)
            xns = scr.tile([128, XG], FP, tag="xns")
            nc.vector.tensor_copy(r(xns[:]), xnp[:])
            for ch in range(4):
                nc.sync.dma_start(r(xft[32:33, XG * ch:XG * ch + XG]),
                                  r(xns[32 * ch:32 * ch + 1, :]))
            nc.sync.dma_start(r(xft[64:98, :]), r(xft[0:34, :]))

        # ---------------- main loop ----------------
        groups = []
        mt = 0
        while mt < MT:
            groups.append(list(range(mt, min(mt + exp_group, MT))))
            mt += exp_group

        with (
            tc.tile_pool(name="gbuf", bufs=2, space="PSUM") as gpool,
            tc.tile_pool(name="accp", bufs=2, space="PSUM") as apool,
            tc.tile_pool(name="ebuf", bufs=3) as epool,
            tc.tile_pool(name="fin", bufs=2) as finp,
        ):
            for ic in range(IC):
                acc = apool.tile([128, ICW], FP, tag="acc")
                for grp in groups:
                    gp = gpool.tile([128, 512 * exp_group], FP, tag="g")
                    for t, mt in enumerate(grp):
                        rg = 64 * (mt % 2)
                        nc.tensor.matmul(
                            gp[:, 512 * t:512 * t + 512],
                            tile_position=(rg, 0),
                            lhsT=r(yft[rg:rg + 34, 128 * mt:128 * mt + 128]),
                            rhs=r(xft[rg:rg + 34, ICW * ic:ICW * ic + ICW]),
                            start=True, stop=True)
                    eb = epool.tile([128, 512 * exp_group], FP, tag="e")
                    w = 512 * len(grp)
                    nc.scalar.activation(r(eb[:, :w]), gp[:, :w], AF.Exp)
                    for t, mt in enumerate(grp):
                        nc.tensor.matmul(
                            acc[0:32, :],
                            tile_position=(0, 0),
                            lhsT=r(zt[:, ZP * mt:ZP * mt + ZP]),
                            rhs=r(eb[:, 512 * t:512 * t + 512]),
                            start=(mt == 0), stop=(mt == MT - 1),
                            skip_group_check=True)
                # fold 4 col-group accumulators via transpose-accumulate
                acc_s = finp.tile([32, ICW], FP, tag="accs")
                nc.vector.tensor_copy(acc_s[:], acc[0:32, :])
                ot = apool.tile([128, 128], FP, tag="acc")
                for q in range(4):
                    nc.tensor.matmul(
                        ot[:, 32 * q:32 * q + 32],
                        tile_position=(0, 0),
                        lhsT=acc_s[0:32, 128 * q:128 * q + 128],
                        rhs=ident[0:32, 0:32],
                        is_transpose=True,
                        start=(q == 0), stop=(q == 3),
                        skip_group_check=True)
                for q in range(4):
                    rec = finp.tile([128, 1], FP, tag="rec")
                    nc.vector.reciprocal(rec[:], ot[:, 32 * q + T:32 * q + T + 1])
                    res = finp.tile([128, T], FP, tag="res")
                    nc.vector.tensor_scalar_mul(res[:], ot[:, 32 * q:32 * q + T],
                                                rec[:])
                    nc.sync.dma_start(
                        OUTd[ICW * ic + 128 * q:ICW * ic + 128 * q + 128, :],
                        res[:])
    nc.compile()
    return nc


def make_in_maps(X, Y, Y_target, W1, b1, W2, b2, W3, b3, n_cores=N_CORES):
    f = lambda a: np.ascontiguousarray(np.asarray(a, dtype=np.float32))
    X, Y, Y_target = f(X), f(Y), f(Y_target)
    W1, W2, W3 = f(W1), f(W2), f(W3)
    b1, b2, b3 = f(b1), f(b2), f(b3)
    m_total = Y.shape[0]
    n_sh = X.shape[0] // n_cores
    Zm = np.zeros((m_total, 32), np.float32)
    Zm[:, :T] = Y_target
    Zm[:, T] = 1.0
    Bs = np.stack([np.tile(b1, 4), np.tile(b2, 4), np.tile(b3, 4)], axis=1)
    common = dict(
        Y=Y, Zm=Zm, W1=W1, W2=W2, W3=W3,
        Bs=np.ascontiguousarray(Bs),
        ident=np.eye(128, dtype=np.float32),
        neghalf=np.full((128, 32), -0.5, np.float32),
        onesrow=np.ones((1, m_total), np.float32),
    )
    return [dict(common, X=X[c * n_sh:(c + 1) * n_sh]) for c in range(n_cores)]


_NC_CACHE = {}


def _get_nc(n_sh, m_total):
    key = (n_sh, m_total)
    if key not in _NC_CACHE:
        use_f32r = os.environ.get("DKR_F32R", "1") == "1"
        _NC_CACHE[key] = build_nc(n_sh, m_total, use_f32r=use_f32r)
    return _NC_CACHE[key]


def kernel(X, Y, Y_target, W1, b1, W2, b2, W3, b3):
    from concourse.bass_utils import run_bass_kernel_spmd

    in_maps = make_in_maps(X, Y, Y_target, W1, b1, W2, b2, W3, b3)
    n_sh = in_maps[0]["X"].shape[0]
    nc = _get_nc(n_sh, np.asarray(Y).shape[0])
    res = run_bass_kernel_spmd(nc, in_maps, core_ids=list(range(N_CORES)))
    return np.concatenate([res.results[c]["out"] for c in range(N_CORES)], axis=0)



# revision 10
# speedup vs baseline: 2941.6473x; 2941.6473x over previous
"""Trainium2 Bass kernel for DeepKernelRegressionModel.

Math (per core, X sharded by rows across 8 cores):
  Xf = MLP(X), Yf = MLP(Y)                        (3-layer relu MLP, H=32)
  K[i,m] = exp(-|Xf_i - Yf_m|^2 / 2)
         = exp(Xf_i . Yf_m - |Xf_i|^2/2 - |Yf_m|^2/2)
  out = (K @ Y_target) / (K @ 1)

All heavy matmuls run in bf16 (1 col/cycle on the PE vs 2 for f32r):
the host pre-transposes X/Y and pre-casts everything to bf16, the MLP
runs on bf16 features, and the kernel-matrix exponent is built from the
*rounded* features so the Gaussian kernel is self-consistent (the large
|Yf|^2 terms cancel exactly against the dot product).  The Y-norm row is
carried in two bf16 rows (hi + lo) for fp32-class accuracy; the X-norm
row is a single bf16 row (its error is constant per output row and
cancels in the weight normalization).

Main loop per i-chunk of 512 X rows: ONE bf16 matmul per 128-row m-tile
produces the exponent (contraction 35 = 32 features + 2 y-norm rows +
1 x-norm row), ScalarE exp's it to bf16, and a second bf16 matmul
contracts with [Y_target, 1] over m.  A final transpose + reciprocal
normalizes.
"""

import numpy as np
from contextlib import ExitStack

import ml_dtypes
import concourse.bass as bass
import concourse.tile as tile
from concourse import bacc, mybir

FP = mybir.dt.float32
FPR = mybir.dt.float32r
BF = mybir.dt.bfloat16
AF = mybir.ActivationFunctionType
BF_NP = ml_dtypes.bfloat16

D, H, T = 64, 32, 8
ZP = 16     # Y_target cols (8) + ones col + zero pad
N_CORES = 8
NROW = 35   # mm1 contraction rows: 32 feat + ynorm hi/lo + xnorm


def _split_matmul_waits(nc):
    """Walrus's S3_LW lowering for self-loading (4-byte) matmuls supports only
    one sync-wait command. Move multi-waits onto a PE sequencer NoOp placed
    right before the matmul — the in-order NX applies them to the stream."""
    import bass_rust

    k = 0
    for fn in nc.m.functions:
        for blk in fn.blocks:
            out = []
            for inst in blk.instructions:
                si = inst.sync_info
                if (type(inst).__name__ == "InstMatmult" and si is not None
                        and si.on_wait and len(si.on_wait) >= 2):
                    waits = list(si.on_wait)
                    for w in waits[:-1]:
                        nop = mybir.InstNoOp(name=f"I-mmwait-{k}", ins=[],
                                             outs=[])
                        k += 1
                        nop.engine = inst.engine
                        nop.sync_info = bass_rust.SyncInfo(
                            on_wait=[w], on_update=[])
                        out.append(nop)
                    inst.sync_info = bass_rust.SyncInfo(
                        on_wait=[waits[-1]], on_update=list(si.on_update))
                out.append(inst)
            blk.instructions = out


def build_nc(n_sh, m_total, exp_group=3, **_ignored):
    """Build the Bass program for one core (SPMD: same program, all cores).

    n_sh: rows of X handled by this core. m_total: rows of Y (full).
    """
    assert n_sh % 512 == 0 and m_total % 2048 == 0
    MT = m_total // 128       # number of 128-row m-tiles
    NCH = m_total // 512      # number of 512-wide m-chunks (MLP)
    CCY = NCH // 4            # stacked col chunks (4 chunks share 128 parts)
    XG = n_sh // 4            # X stacked-chunk width
    IC = n_sh // 512          # i-chunks
    ICW = 512

    def r(ap):
        return ap.bitcast(FPR)

    nc = bacc.Bacc("TRN2", target_bir_lowering=False, debug=False,
                   num_devices=N_CORES)

    XTd = nc.dram_tensor("XT", [D, n_sh], BF, kind="ExternalInput").ap()
    YTd = nc.dram_tensor("YT", [D, m_total], BF, kind="ExternalInput").ap()
    ZTd = nc.dram_tensor("ZT", [128, MT * ZP], BF, kind="ExternalInput").ap()
    W1d = nc.dram_tensor("W1", [D, H], BF, kind="ExternalInput").ap()
    W2d = nc.dram_tensor("W2", [H, H], BF, kind="ExternalInput").ap()
    W3d = nc.dram_tensor("W3", [H, H], BF, kind="ExternalInput").ap()
    Bd = nc.dram_tensor("Bs", [128, 3], FP, kind="ExternalInput").ap()
    Id = nc.dram_tensor("ident", [128, 128], BF, kind="ExternalInput").ap()
    Ifd = nc.dram_tensor("identf", [ZP, ZP], FP, kind="ExternalInput").ap()
    NHd = nc.dram_tensor("neghalf", [128, 2], FP, kind="ExternalInput").ap()
    ORd = nc.dram_tensor("onesrow", [2, m_total], BF, kind="ExternalInput").ap()
    OUTd = nc.dram_tensor("out", [n_sh, T], FP, kind="ExternalOutput").ap()

    with tile.TileContext(nc) as tc, ExitStack() as ctx:
        const = ctx.enter_context(tc.tile_pool(name="const", bufs=1))
        big = ctx.enter_context(tc.tile_pool(name="big", bufs=1))
        scr = ctx.enter_context(tc.tile_pool(name="scr", bufs=1))

        w1s = const.tile([D, H], BF)
        nc.sync.dma_start(w1s[:], W1d[:])
        w2s = const.tile([128, H], BF)
        w3s = const.tile([128, H], BF)
        for g in range(4):
            nc.sync.dma_start(w2s[32 * g:32 * g + 32, :], W2d[:])
            nc.sync.dma_start(w3s[32 * g:32 * g + 32, :], W3d[:])
        bs = const.tile([128, 3], FP)
        nc.sync.dma_start(bs[:], Bd[:])
        ident = const.tile([128, 128], BF)
        nc.sync.dma_start(ident[:], Id[:])
        identf = const.tile([ZP, ZP], FP)
        nc.sync.dma_start(identf[:], Ifd[:])
        nh = const.tile([128, 2], FP)
        nc.sync.dma_start(nh[:].bitcast(FPR), NHd[:].bitcast(FPR))
        zt = const.tile([128, MT * ZP], BF)
        for g in range(4):
            w = MT * ZP // 4
            nc.sync.dma_start(zt[:, w * g:w * g + w], ZTd[:, w * g:w * g + w])

        # persistent big tensors
        yTs = big.tile([D, m_total], BF)     # Y^T (host pre-transposed)
        xTs = big.tile([D, n_sh], BF)        # X^T
        yft = big.tile([128, m_total], BF)   # rows 0-34 aug A, 64-98 aug B
        xft = big.tile([128, n_sh], BF)
        yfs = big.tile([128, 512 * CCY], BF)   # Y features, stacked
        sqy = big.tile([128, 512 * CCY], FP)   # squared features (exact)

        for ch in range(NCH):
            nc.sync.dma_start(yTs[:, 512 * ch:512 * ch + 512],
                              YTd[:, 512 * ch:512 * ch + 512])
        nc.sync.dma_start(xTs[:], XTd[:])

        # ---------------- phase Y: MLP (stacked 4x) ----------------
        # chunk ch (512 m's) -> partition group cg = ch%4, col chunk cc = ch//4
        # processed in blocks of 8 chunks so psum tiles stay at 2 banks
        with (
            tc.tile_pool(name="mlp_psum", bufs=2, space="PSUM") as mpp,
            tc.tile_pool(name="np_psum", bufs=1, space="PSUM") as npp,
            tc.tile_pool(name="acts", bufs=2) as actp,
        ):
            NBLK = (NCH + 7) // 8
            for blk in range(NBLK):
                chs = list(range(8 * blk, min(8 * blk + 8, NCH)))
                ccs = sorted({ch // 4 for ch in chs})
                c0, w = ccs[0], 512 * len(ccs)
                h1p = mpp.tile([128, w], FP, tag="hp")
                for ch in chs:
                    cg, lc = ch % 4, ch // 4 - c0
                    nc.tensor.matmul(h1p[32 * cg:32 * cg + 32, 512 * lc:512 * lc + 512],
                                     lhsT=w1s[:], rhs=yTs[:, 512 * ch:512 * ch + 512],
                                     start=True, stop=True,
                                     skip_group_check=True,
                                     tile_position=(0, 32 * cg))
                h1s = actp.tile([128, w], BF, tag="hs")
                for lc in range(len(ccs)):
                    nc.scalar.activation(h1s[:, 512 * lc:512 * lc + 512],
                                         h1p[:, 512 * lc:512 * lc + 512],
                                         AF.Relu, bias=bs[:, 0:1])
                h2p = mpp.tile([128, w], FP, tag="hp")
                for ch in chs:
                    cg, lc = ch % 4, ch // 4 - c0
                    nc.tensor.matmul(h2p[32 * cg:32 * cg + 32, 512 * lc:512 * lc + 512],
                                     tile_position=(32 * cg, 32 * cg),
                                     lhsT=w2s[32 * cg:32 * cg + 32, :],
                                     rhs=h1s[32 * cg:32 * cg + 32, 512 * lc:512 * lc + 512],
                                     start=True, stop=True,
                                     skip_group_check=True)
                h2s = actp.tile([128, w], BF, tag="hs")
                for lc in range(len(ccs)):
                    nc.scalar.activation(h2s[:, 512 * lc:512 * lc + 512],
                                         h2p[:, 512 * lc:512 * lc + 512],
                                         AF.Relu, bias=bs[:, 1:2])
                h3p = mpp.tile([128, w], FP, tag="hp")
                for ch in chs:
                    cg, lc = ch % 4, ch // 4 - c0
                    nc.tensor.matmul(h3p[32 * cg:32 * cg + 32, 512 * lc:512 * lc + 512],
                                     tile_position=(32 * cg, 32 * cg),
                                     lhsT=w3s[32 * cg:32 * cg + 32, :],
                                     rhs=h2s[32 * cg:32 * cg + 32, 512 * lc:512 * lc + 512],
                                     start=True, stop=True,
                                     skip_group_check=True)
                for lc, cc in enumerate(ccs):
                    nc.scalar.activation(yfs[:, 512 * cc:512 * cc + 512],
                                         h3p[:, 512 * lc:512 * lc + 512],
                                         AF.Relu, bias=bs[:, 2:3])
                    nc.vector.tensor_mul(sqy[:, 512 * cc:512 * cc + 512].bitcast(FPR),
                                         yfs[:, 512 * cc:512 * cc + 512],
                                         yfs[:, 512 * cc:512 * cc + 512])
            # assemble yft rows 0-31 (flat layout)
            for ch in range(NCH):
                cg, cc = ch % 4, ch // 4
                nc.sync.dma_start(yft[0:32, 512 * ch:512 * ch + 512],
                                  yfs[32 * cg:32 * cg + 32, 512 * cc:512 * cc + 512])
            nc.sync.dma_start(yft[34:35, :], ORd[0:1, :])  # ones row

            # ---- Y norms: ynp[p, 2mt:2mt+2] = -|Yf_{128mt+p}|^2/2 (dup) ----
            # (f32r matmuls need even moving free size + even psum offsets)
            ynp = npp.tile([128, 2 * MT], FP, tag="ynp")
            for mt in range(MT):
                ch, s = mt // 4, mt % 4
                cg, cc = ch % 4, ch // 4
                col = 512 * cc + 128 * s
                nc.tensor.matmul(ynp[:, 2 * mt:2 * mt + 2],
                                 tile_position=(32 * cg, 0),
                                 lhsT=r(sqy[32 * cg:32 * cg + 32, col:col + 128]),
                                 rhs=r(nh[32 * cg:32 * cg + 32, :]),
                                 start=True, stop=True,
                                 skip_group_check=True)
            ynhi = scr.tile([128, 2 * MT], BF, tag="ynhi")
            ynlo = scr.tile([128, 2 * MT], BF, tag="ynlo")
            nc.vector.tensor_copy(ynhi[:], ynp[:])
            nc.vector.tensor_sub(ynlo[:], ynp[:], ynhi[:])
            # transpose [128, 2MT] -> [2MT, 128] so row 2mt is m-tile mt's
            # norm span, then DMA rows into yft rows 32/33
            ytr = npp.tile([2 * MT, 256], BF, tag="ytr")
            nc.tensor.matmul(ytr[:, 0:128], lhsT=ynhi[:], rhs=ident[:],
                             is_transpose=True, start=True, stop=False,
                             skip_group_check=True)
            nc.tensor.matmul(ytr[:, 128:256], lhsT=ynlo[:], rhs=ident[:],
                             is_transpose=True, start=False, stop=True,
                             skip_group_check=True)
            ytrs = scr.tile([2 * MT, 256], BF, tag="ytrs")
            nc.vector.tensor_copy(ytrs[:], ytr[:])
            for mt in range(MT):
                nc.sync.dma_start(yft[32:33, 128 * mt:128 * mt + 128],
                                  ytrs[2 * mt:2 * mt + 1, 0:128])
                nc.sync.dma_start(yft[33:34, 128 * mt:128 * mt + 128],
                                  ytrs[2 * mt:2 * mt + 1, 128:256])
            # duplicate augmented block to partitions 64-98 (row group B)
            for sg in range(4):
                seg = m_total // 4
                nc.sync.dma_start(yft[64:64 + NROW, seg * sg:seg * sg + seg],
                                  yft[0:NROW, seg * sg:seg * sg + seg])

            # ---------------- phase X: MLP (4 chunks, stacked) ----------------
            hx1 = mpp.tile([128, XG], FP, tag="hp")
            for g in range(4):
                nc.tensor.matmul(hx1[32 * g:32 * g + 32, :],
                                 tile_position=(0, 32 * g),
                                 lhsT=w1s[:],
                                 rhs=xTs[:, XG * g:XG * g + XG],
                                 start=True, stop=True,
                                 skip_group_check=True)
            hx1s = scr.tile([128, XG], BF, tag="hxs1")
            nc.scalar.activation(hx1s[:], hx1[:], AF.Relu, bias=bs[:, 0:1])
            hx2 = mpp.tile([128, XG], FP, tag="hp")
            for g in range(4):
                nc.tensor.matmul(hx2[32 * g:32 * g + 32, :],
                                 tile_position=(32 * g, 32 * g),
                                 lhsT=w2s[32 * g:32 * g + 32, :],
                                 rhs=hx1s[32 * g:32 * g + 32, :],
                                 start=True, stop=True,
                                 skip_group_check=True)
            hx2s = scr.tile([128, XG], BF, tag="hxs2")
            nc.scalar.activation(hx2s[:], hx2[:], AF.Relu, bias=bs[:, 1:2])
            hx3 = mpp.tile([128, XG], FP, tag="hp")
            for g in range(4):
                nc.tensor.matmul(hx3[32 * g:32 * g + 32, :],
                                 tile_position=(32 * g, 32 * g),
                                 lhsT=w3s[32 * g:32 * g + 32, :],
                                 rhs=hx2s[32 * g:32 * g + 32, :],
                                 start=True, stop=True,
                                 skip_group_check=True)
            xfs = scr.tile([128, XG], BF, tag="xfs")
            nc.scalar.activation(xfs[:], hx3[:], AF.Relu, bias=bs[:, 2:3])
            sqx = scr.tile([128, XG], FP, tag="sqx")
            nc.vector.tensor_mul(sqx[:].bitcast(FPR), xfs[:], xfs[:])
            for g in range(4):
                nc.sync.dma_start(xft[0:32, XG * g:XG * g + XG],
                                  xfs[32 * g:32 * g + 32, :])
            nc.sync.dma_start(xft[32:34, 0:n_sh], ORd[0:2, 0:n_sh])  # ones rows
            # X norms -> single bf16 row 34
            IT = n_sh // 128
            xnp = npp.tile([128, 2 * IT], FP, tag="ynp")
            for it in range(IT):
                g, s = it // (XG // 128), it % (XG // 128)
                col = 128 * s
                nc.tensor.matmul(xnp[:, 2 * it:2 * it + 2],
                                 tile_position=(32 * g, 0),
                                 lhsT=r(sqx[32 * g:32 * g + 32, col:col + 128]),
                                 rhs=r(nh[32 * g:32 * g + 32, :]),
                                 start=True, stop=True,
                                 skip_group_check=True)
            xnhi = scr.tile([128, 2 * IT], BF, tag="xnhi")
            nc.vector.tensor_copy(xnhi[:], xnp[:])
            xtr = npp.tile([2 * IT, 128], BF, tag="ytr")
            nc.tensor.matmul(xtr[:], lhsT=xnhi[:], rhs=ident[:],
                             is_transpose=True, start=True, stop=True,
                             skip_group_check=True)
            xtrs = scr.tile([2 * IT, 128], BF, tag="xtrs")
            nc.vector.tensor_copy(xtrs[:], xtr[:])
            for it in range(IT):
                nc.sync.dma_start(xft[34:35, 128 * it:128 * it + 128],
                                  xtrs[2 * it:2 * it + 1, :])
            nc.sync.dma_start(xft[64:64 + NROW, :], xft[0:NROW, :])

        # ---------------- main loop ----------------
        groups = []
        mt = 0
        while mt < MT:
            groups.append(list(range(mt, min(mt + exp_group, MT))))
            mt += exp_group

        with (
            tc.tile_pool(name="gbuf", bufs=2, space="PSUM") as gpool,
            tc.tile_pool(name="accp", bufs=2, space="PSUM") as apool,
            tc.tile_pool(name="ebuf", bufs=3) as epool,
            tc.tile_pool(name="fin", bufs=2) as finp,
        ):
            for ic in range(IC):
                acc = apool.tile([128, ICW], FP, tag="acc")
                for grp in groups:
                    gp = gpool.tile([128, 512 * exp_group], FP, tag="g")
                    for t, mt in enumerate(grp):
                        rg = 64 * (mt % 2)
                        nc.tensor.matmul(
                            gp[:, 512 * t:512 * t + 512],
                            tile_position=(rg, 0),
                            lhsT=yft[rg:rg + NROW, 128 * mt:128 * mt + 128],
                            rhs=xft[rg:rg + NROW, ICW * ic:ICW * ic + ICW],
                            start=True, stop=True)
                    eb = epool.tile([128, 512 * exp_group], BF, tag="e")
                    w = 512 * len(grp)
                    nc.scalar.activation(eb[:, :w], gp[:, :w], AF.Exp)
                    for t, mt in enumerate(grp):
                        nc.tensor.matmul(
                            acc[0:ZP, :],
                            tile_position=(0, 0),
                            lhsT=zt[:, ZP * mt:ZP * mt + ZP],
                            rhs=eb[:, 512 * t:512 * t + 512],
                            start=(mt == 0), stop=(mt == MT - 1),
                            skip_group_check=True)
                # fold 4 col-group accumulators via transpose-accumulate
                acc_s = finp.tile([ZP, ICW], FP, tag="accs")
                nc.vector.tensor_copy(acc_s[:], acc[0:ZP, :])
                ot = apool.tile([128, 4 * ZP], FP, tag="acc")
                for q in range(4):
                    nc.tensor.matmul(
                        ot[:, ZP * q:ZP * q + ZP],
                        tile_position=(0, 0),
                        lhsT=acc_s[0:ZP, 128 * q:128 * q + 128],
                        rhs=identf[:],
                        is_transpose=True,
                        start=(q == 0), stop=(q == 3),
                        skip_group_check=True)
                for q in range(4):
                    rec = finp.tile([128, 1], FP, tag="rec")
                    nc.vector.reciprocal(rec[:], ot[:, ZP * q + T:ZP * q + T + 1])
                    res = finp.tile([128, T], FP, tag="res")
                    nc.vector.tensor_scalar_mul(res[:], ot[:, ZP * q:ZP * q + T],
                                                rec[:])
                    nc.sync.dma_start(
                        OUTd[ICW * ic + 128 * q:ICW * ic + 128 * q + 128, :],
                        res[:])
    nc.compile()
    return nc


def make_in_maps(X, Y, Y_target, W1, b1, W2, b2, W3, b3, n_cores=N_CORES):
    f32 = lambda a: np.ascontiguousarray(np.asarray(a, dtype=np.float32))
    bf = lambda a: np.ascontiguousarray(np.asarray(a, dtype=np.float32).astype(BF_NP))
    X, Y, Y_target = f32(X), f32(Y), f32(Y_target)
    b1, b2, b3 = f32(b1), f32(b2), f32(b3)
    m_total = Y.shape[0]
    n_sh = X.shape[0] // n_cores
    MT = m_total // 128
    Zm = np.zeros((m_total, ZP), np.float32)
    Zm[:, :T] = Y_target
    Zm[:, T] = 1.0
    # pre-tiled [128, MT*ZP]: ZT[p, mt*ZP + c] = Zm[128*mt + p, c]
    ZT = np.transpose(Zm.reshape(MT, 128, ZP), (1, 0, 2)).reshape(128, MT * ZP)
    Bs = np.stack([np.tile(b1, 4), np.tile(b2, 4), np.tile(b3, 4)], axis=1)
    common = dict(
        YT=bf(Y.T), ZT=bf(ZT), W1=bf(W1), W2=bf(W2), W3=bf(W3),
        Bs=np.ascontiguousarray(Bs),
        ident=bf(np.eye(128, dtype=np.float32)),
        identf=np.eye(ZP, dtype=np.float32),
        neghalf=np.full((128, 2), -0.5, np.float32),
        onesrow=np.ones((2, m_total), BF_NP),
    )
    return [dict(common, XT=bf(X[c * n_sh:(c + 1) * n_sh].T))
            for c in range(n_cores)]


_NC_CACHE = {}


def _get_nc(n_sh, m_total):
    key = (n_sh, m_total)
    if key not in _NC_CACHE:
        _NC_CACHE[key] = build_nc(n_sh, m_total)
    return _NC_CACHE[key]


def kernel(X, Y, Y_target, W1, b1, W2, b2, W3, b3):
    from concourse.bass_utils import run_bass_kernel_spmd

    in_maps = make_in_maps(X, Y, Y_target, W1, b1, W2, b2, W3, b3)
    n_sh = in_maps[0]["XT"].shape[1]
    nc = _get_nc(n_sh, np.asarray(Y).shape[0])
    res = run_bass_kernel_spmd(nc, in_maps, core_ids=list(range(N_CORES)))
    return np.concatenate([res.results[c]["out"] for c in range(N_CORES)], axis=0)
